# revision 1
# baseline (speedup 1.0000x reference)
"""Trainium2 Bass kernel for nn_MixNode (soft decision tree / MoE routing).

The recursive MixNode tree collapses algebraically:
    out[b] = sum_m C_m(x_b) * leafG[m]
where leafG folds the (input-independent) gamma-softmax products into the
leaf vectors, and C_m = prod of routing probabilities along the root->m
path. With delta = logit0 - logit1 per internal node, the two routing
probs are sigmoid(+-delta), so log C = A @ (-softplus(-+delta)) for a
constant 0/-1 path matrix A. softplus(z) = ln(exp(z) + 1) is computed as
Exp then Ln(x + 1); both +-delta blocks are produced by one doubled
matmul (weights [-Wd; +Wd]) so each chunk needs only one Exp + one Ln.

Device pipeline per core (batch shard 2048 rows, 4 chunks of 512):
    DMA x -> PE transpose (feature-major) -> mm1 D2 = [-Wd;+Wd] @ x^T
    -> ACT exp -> ACT ln1p -> sp -> mmA S = A @ sp -> ACT exp -> C
    -> mm2 out = (C-slices)^T @ leafG -> DMA out.
All matmuls are exact fp32 (fp32r would run 4x faster on the PE but
measures ~4e-4 absmax error vs the reference's 1e-5). Exp and Ln are
pinned to the one ACT table set containing both, so the whole kernel
performs a single activation-table load (~2.7us per reload otherwise).

Sharding: pure data parallelism over the batch dim across 8 cores;
the small tree parameters are folded host-side and replicated.
"""

import os
import sys

import numpy as np

for _p in ("/opt/trn_rl_repo", "/root/.axon_site/_ro/trn_rl_repo"):
    if os.path.isdir(_p) and _p not in sys.path:
        sys.path.append(_p)

import concourse.tile as tile
from concourse import bacc, mybir
from concourse.bass_utils import run_bass_kernel_spmd

N_CORES = 8
BATCH, D_IN, D_OUT = 16384, 512, 128
B_CORE = BATCH // N_CORES  # 2048
N_INT, N_ALL = 31, 63
P = 128
NCH = 4  # batch chunks per core, 512 rows each
SC = 4   # 128-row subtiles per chunk
KC = 4   # 128-feature chunks

F32 = mybir.dt.float32
F32R = mybir.dt.float32r
AF = mybir.ActivationFunctionType

# float32r matmuls run the PE at full rate (vs 4 cycles/row for exact fp32)
# at reduced multiply precision. Overridable for experiments.
MM_FAST = os.environ.get("MIX_MM_FAST", "0") == "1"


def _emit(nc, mm_fast: bool):
    x_d = nc.dram_tensor("x", [B_CORE, D_IN], F32, kind="ExternalInput")
    wdT_d = nc.dram_tensor("wdT", [KC, P, 64], F32, kind="ExternalInput")
    aT_d = nc.dram_tensor("aT", [64, N_ALL], F32, kind="ExternalInput")
    leafG_d = nc.dram_tensor("leafG", [N_ALL, D_OUT], F32, kind="ExternalInput")
    biasN_d = nc.dram_tensor("biasN", [64, 1], F32, kind="ExternalInput")
    ident_d = nc.dram_tensor("ident", [P, P], F32, kind="ExternalInput")
    y_d = nc.dram_tensor("y", [B_CORE, D_OUT], F32, kind="ExternalOutput")

    # float32r matmul inputs must be produced as float32r (HW rounds the
    # mantissa at the producer); constants get a one-time rounding copy.
    mdt = F32R if mm_fast else F32

    with tile.TileContext(nc) as tc:
        with (
            tc.tile_pool(name="const", bufs=1) as constp,
            tc.tile_pool(name="xin", bufs=4) as xinp,
            tc.tile_pool(name="xtp", bufs=2) as xtpp,
            tc.tile_pool(name="act", bufs=4) as actp,
            tc.tile_pool(name="spc", bufs=4) as spp,
            tc.tile_pool(name="ccp", bufs=4) as ccp,
            tc.tile_pool(name="osbp", bufs=2) as osbp,
            tc.tile_pool(name="tps", bufs=3, space="PSUM") as tpsp,
            tc.tile_pool(name="dps", bufs=2, space="PSUM") as dpsp,
            tc.tile_pool(name="sps", bufs=1, space="PSUM") as spsp,
            tc.tile_pool(name="ops", bufs=2, space="PSUM") as opsp,
        ):
            # Prefetch the full x shard first: the sync queue issues these
            # before anything else so the PE pipeline starts ASAP.
            xin_l = []
            for c in range(NCH):
                xin = xinp.tile([P, SC, D_IN], F32, tag="xin")
                if c == 0:
                    # quarter loads: the first transpose starts after 256KB
                    for s in range(SC):
                        r0 = s * P
                        nc.sync.dma_start(xin[:, s, :], x_d[r0:r0 + P, :])
                else:
                    for h in range(2):
                        xsrc = x_d[c * 512 + h * 256:
                                   c * 512 + (h + 1) * 256, :]
                        nc.sync.dma_start(
                            xin[:, h * 2:(h + 1) * 2, :],
                            xsrc.rearrange("(s p) f -> p s f", p=P))
                xin_l.append(xin)

            ident = constp.tile([P, P], F32)
            nc.gpsimd.dma_start(ident[:], ident_d[:])
            # PE warm-up: dummy matmuls on a zeroed scratch tile (gated
            # only on the local memset, not the slow ident DMA) flip the
            # HAM clock gate to 2.4GHz before the real transposes arrive.
            warm = constp.tile([P, P], F32)
            nc.gpsimd.memset(warm[:], 0.0)
            wps = tpsp.tile([P, 512], F32, tag="tps")
            for w in range(12):
                nc.tensor.matmul(
                    wps[:, (w % 4) * P:(w % 4 + 1) * P], warm[:], warm[:],
                    start=(w % 4 == 0), stop=(w % 4 == 3))
            wdT0 = constp.tile([P, KC, 64], F32)
            for k in range(KC):
                nc.gpsimd.dma_start(wdT0[:, k, :], wdT_d[k])
            aT0 = constp.tile([64, N_ALL], F32)
            nc.gpsimd.dma_start(aT0[:], aT_d[:])
            leafG0 = constp.tile([N_ALL, D_OUT], F32)
            nc.gpsimd.dma_start(leafG0[:], leafG_d[:])
            biasN = constp.tile([64, 1], F32)
            nc.gpsimd.dma_start(biasN[:], biasN_d[:])
            if mm_fast:
                wdT = constp.tile([P, KC, 64], F32R)
                nc.vector.tensor_copy(wdT[:], wdT0[:])
                aT = constp.tile([64, N_ALL], F32R)
                nc.vector.tensor_copy(aT[:], aT0[:])
                leafG = constp.tile([N_ALL, D_OUT], F32R)
                nc.vector.tensor_copy(leafG[:], leafG0[:])
            else:
                wdT, aT, leafG = wdT0, aT0, leafG0

            # Single ACT table set (natural_log_exp_and_others, pinned in
            # build()) lets Exp/Ln interleave freely with one table load.
            for c in range(NCH):
                xin = xin_l[c]

                # xT[k] = x-chunk^T (feature-major), [128f, 512b]
                xT = xtpp.tile([P, KC, 512], mdt)
                for k in range(KC):
                    tps = tpsp.tile([P, 512], F32)
                    for s in range(SC):
                        nc.tensor.matmul(
                            tps[:, s * P:(s + 1) * P],
                            xin[:, s, k * P:(k + 1) * P],
                            ident[:],
                            is_transpose=True,
                            start=(s == 0),
                            stop=(s == SC - 1),
                        )
                    nc.vector.tensor_copy(xT[:, k, :], tps[:])

                # mm1: D2 = [-Wd; +Wd] @ x^T [64, 512], rows 31/63 = 0 (pad)
                dps = dpsp.tile([64, 512], F32)
                for k in range(KC):
                    nc.tensor.matmul(
                        dps[:],
                        wdT[:, k, :],
                        xT[:, k, :],
                        start=(k == 0),
                        stop=(k == KC - 1),
                    )

                # t = exp(D2 + bias) = [exp(-delta); exp(+delta)];
                # sp = ln(t + 1) = softplus blocks (pad rows ln2, nulled by
                # the zero rows of A).
                t = actp.tile([64, 512], F32, tag="texp")
                nc.scalar.activation(t[:], dps[:], AF.Exp, bias=biasN[:])
                sp = spp.tile([64, 512], mdt, tag="sp")
                nc.scalar.activation(sp[:], t[:], AF.Ln, bias=1.0)

                # S = A @ sp (log path probs), C = exp(S)
                sps = spsp.tile([N_ALL, 512], F32)
                nc.tensor.matmul(sps[:], aT[:], sp[:], start=True, stop=True)
                cc = ccp.tile([N_ALL, 512], mdt, tag="cc")
                nc.scalar.activation(cc[:], sps[:], AF.Exp)

                # mm2: out[s] = C[:, s-block]^T @ leafG, in half-chunks on
                # separate PSUM banks so copy/store overlap the next half.
                osb = osbp.tile([P, SC, D_OUT], F32)
                for h in range(2):
                    ops = opsp.tile([P, 256], F32, tag="ops")
                    for si in range(2):
                        s = 2 * h + si
                        nc.tensor.matmul(
                            ops[:, si * P:(si + 1) * P],
                            cc[:, s * P:(s + 1) * P],
                            leafG[:],
                            start=(si == 0),
                            stop=(si == 1),
                        )
                    nc.scalar.copy(
                        osb[:, 2 * h:2 * h + 2, :].rearrange(
                            "p s o -> p (s o)"),
                        ops[:])
                    ydst = y_d[c * 512 + h * 256:
                               c * 512 + (h + 1) * 256, :].rearrange(
                        "(s p) o -> p s o", p=P)
                    nc.sync.dma_start(ydst, osb[:, 2 * h:2 * h + 2, :])
    return nc


_BUILD_CACHE = {}


def _pin_act_tables(nc):
    """Restrict Exp/Ln to the one table set that holds both, so the
    table-load placement pass emits a single ACT_TABLE_LOAD instead of
    thrashing between per-function sets (~2.7us per switch)."""
    from concourse import hw_specs
    tables = hw_specs.get_activation_tables(nc.m.arch)
    both = "natural_log_exp_and_others"
    if both in tables and AF.Exp in tables[both] and AF.Ln in tables[both]:
        for name, fns in tables.items():
            if name != both:
                fns.discard(AF.Exp)
                fns.discard(AF.Ln)


def build(mm_fast: bool = MM_FAST):
    key = bool(mm_fast)
    if key not in _BUILD_CACHE:
        nc = bacc.Bacc("TRN2", target_bir_lowering=False, debug=False,
                       num_devices=N_CORES)
        _pin_act_tables(nc)
        _emit(nc, mm_fast)
        nc.compile()
        _BUILD_CACHE[key] = nc
    return _BUILD_CACHE[key]


def host_prep(W, b, gamma, leaf):
    """Fold the tiny tree parameters into the kernel's constant tensors."""
    W = np.asarray(W, np.float32)
    b = np.asarray(b, np.float32)
    gamma = np.asarray(gamma, np.float32)
    leaf = np.asarray(leaf, np.float32)

    Wd = W[:, 0, :] - W[:, 1, :]                      # [31, 512]
    bd = b[:, 0] - b[:, 1]                            # [31]
    e = np.exp(gamma - gamma.max(-1, keepdims=True))
    g = e / e.sum(-1, keepdims=True)                  # [31, 2]

    path = np.zeros(N_ALL, np.float64)
    path[0] = 1.0
    for m in range(1, N_ALL):
        par = (m - 1) // 2
        path[m] = path[par] * g[par, 0]
    G = np.array([path[m] * (g[m, 1] if m < N_INT else 1.0)
                  for m in range(N_ALL)])
    leafG = (G[:, None] * leaf.astype(np.float64)).astype(np.float32)

    # A[row, m] = -1 if the edge lives on the root->m path.
    # Edge (node a, child j) -> row a (j=0) or row 32+a (j=1); rows 31/63 pad.
    A = np.zeros((64, N_ALL), np.float32)
    for m in range(N_ALL):
        node = m
        while node:
            par = (node - 1) // 2
            j = node - 2 * par - 1
            A[par if j == 0 else 32 + par, m] = -1.0
            node = par

    # Doubled routing weights: cols 0..30 = -Wd^T, cols 32..62 = +Wd^T.
    wdT = np.zeros((KC, P, 64), np.float32)
    wdTfull = np.ascontiguousarray(Wd.T)              # [512, 31]
    for k in range(KC):
        blk = wdTfull[k * P:(k + 1) * P]
        wdT[k, :, 0:N_INT] = -blk
        wdT[k, :, 32:32 + N_INT] = blk

    biasN = np.zeros((64, 1), np.float32)
    biasN[0:N_INT, 0] = -bd
    biasN[32:32 + N_INT, 0] = bd
    ident = np.eye(P, dtype=np.float32)
    return {
        "wdT": wdT,
        "aT": np.ascontiguousarray(A),
        "leafG": np.ascontiguousarray(leafG),
        "biasN": biasN,
        "ident": ident,
    }


def run(x, W, b, gamma, leaf, mm_fast: bool = MM_FAST, **spmd_kwargs):
    x = np.asarray(x, np.float32)
    consts = host_prep(W, b, gamma, leaf)
    shards = x.reshape(N_CORES, B_CORE, D_IN)
    in_maps = [dict(consts, x=np.ascontiguousarray(shards[i]))
               for i in range(N_CORES)]
    nc = build(mm_fast)
    res = run_bass_kernel_spmd(nc, in_maps, list(range(N_CORES)), **spmd_kwargs)
    y = np.concatenate([res.results[i]["y"] for i in range(N_CORES)], axis=0)
    return y, res


def kernel(x, W, b, gamma, leaf):
    y, _ = run(x, W, b, gamma, leaf)
    return y



# revision 8
# speedup vs baseline: 1.4267x; 1.4267x over previous
"""Trainium2 Bass kernel for nn_MixNode (soft decision tree / MoE routing).

The recursive MixNode tree collapses algebraically:
    out[b] = sum_m C_m(x_b) * leafG[m]
where leafG folds the (input-independent) gamma-softmax products into the
leaf vectors, and C_m = prod of routing probabilities along the root->m
path. With delta = logit0 - logit1 per internal node, the two routing
probs are sigmoid(+-delta), so log C = A @ (-softplus(-+delta)) for a
constant 0/-1 path matrix A. softplus(z) = ln(exp(z) + 1) is computed as
Exp then Ln(x + 1); both +-delta blocks are produced by one doubled
matmul (weights [-Wd; +Wd]) so each chunk needs only one Exp + one Ln.

Device pipeline per core (batch shard 2048 rows, 4 chunks of 512):
    DMA x -> PE transpose (feature-major, fp32r 1.5 cyc/row) ->
    mm1 D2 = [-Wd;+Wd] @ x^T -> ACT exp -> ACT ln1p -> sp ->
    mmA S = A @ sp -> ACT exp -> C -> mm2 out^T = leafG^T @ C
    -> Pool copy -> DMA out.
All matmuls run in float32r (full-rate PE, ~1e-4 rel err vs fp32 -- well
under the 2e-2 gate). The output is produced output-major [128, 2048]
per core so mm2 is a single 512-wide matmul per chunk and the store DMA
uses 2 KiB descriptors; the host transposes when gathering.
Exp and Ln are pinned to the one ACT table set containing both, so the
whole kernel performs a single activation-table load.

DMA: all four x-chunk loads are single instructions on the sync queue
(512 x 2 KiB descriptors each) issued up front; the packed constant
block (ident | [-Wd;+Wd]^T | A | bias | leafG) is one DMA on the scalar
queue; y stores go back on the sync queue behind the loads.

Sharding: pure data parallelism over the batch dim across 8 cores;
the small tree parameters are folded host-side and replicated.
"""

import os
import sys

import numpy as np

for _p in ("/opt/trn_rl_repo", "/root/.axon_site/_ro/trn_rl_repo"):
    if os.path.isdir(_p) and _p not in sys.path:
        sys.path.append(_p)

import concourse.tile as tile
from concourse import bacc, mybir
from concourse.bass_utils import run_bass_kernel_spmd

N_CORES = 8
BATCH, D_IN, D_OUT = 16384, 512, 128
B_CORE = BATCH // N_CORES  # 2048
N_INT, N_ALL = 31, 63
P = 128
NCH = 4  # batch chunks per core, 512 rows each
SC = 4   # 128-row subtiles per chunk
KC = 4   # 128-feature chunks
PKW = 128 + 256 + 63 + 1 + 128  # packed const width = 576

F32 = mybir.dt.float32
F32R = mybir.dt.float32r
AF = mybir.ActivationFunctionType

N_WARM = 8  # PE clock-ramp matmuls before the first transpose


def _emit(nc):
    x_d = nc.dram_tensor("x", [B_CORE, D_IN], F32R, kind="ExternalInput")
    pk_d = nc.dram_tensor("pk", [P, PKW], F32R, kind="ExternalInput")
    y_d = nc.dram_tensor("y", [D_OUT, B_CORE], F32, kind="ExternalOutput")

    with tile.TileContext(nc) as tc:
        with (
            tc.tile_pool(name="const", bufs=1) as constp,
            tc.tile_pool(name="xin", bufs=4) as xinp,
            tc.tile_pool(name="xtp", bufs=2) as xtpp,
            tc.tile_pool(name="act", bufs=2) as actp,
            tc.tile_pool(name="spc", bufs=2) as spp,
            tc.tile_pool(name="ccp", bufs=2) as ccp,
            tc.tile_pool(name="osbp", bufs=2) as osbp,
            tc.tile_pool(name="tps", bufs=3, space="PSUM") as tpsp,
            tc.tile_pool(name="dps", bufs=2, space="PSUM") as dpsp,
            tc.tile_pool(name="sps", bufs=1, space="PSUM") as spsp,
            tc.tile_pool(name="ops", bufs=2, space="PSUM") as opsp,
        ):
            # Prefetch the full x shard: one DMA instruction per 512-row
            # chunk on the sync queue (512 descriptors of 2 KiB each),
            # issued before everything else so transfers start ASAP.
            xin_l = []
            for c in range(NCH):
                xin = xinp.tile([P, SC, D_IN], F32R, tag="xin")
                nc.sync.dma_start(
                    xin[:],
                    x_d[c * 512:(c + 1) * 512, :].rearrange(
                        "(s p) f -> p s f", p=P))
                xin_l.append(xin)

            # Packed constants in a single DMA on the scalar queue. The
            # tile is float32r because the BIR verifier requires fp32r
            # matmul inputs to be *produced* as fp32r; the ACT bias slice
            # is bitcast back (same bits either way).
            pk = constp.tile([P, PKW], F32R)
            nc.scalar.dma_start(pk[:], pk_d[:])
            ident = pk[:, 0:128]
            wdT = [pk[:, 128 + 64 * k:128 + 64 * (k + 1)]
                   for k in range(KC)]
            aT = pk[0:64, 384:447]
            biasN = pk[0:64, 447:448].bitcast(F32)
            leafG = pk[0:63, 448:576]

            # PE warm-up: dummy fp32 matmuls on a zeroed scratch tile flip
            # the HAM clock gate to 2.4GHz and keep the PE busy until the
            # first x chunk lands (~4.5us).
            warm = constp.tile([P, P], F32)
            nc.gpsimd.memset(warm[:], 0.0)
            wps = opsp.tile([P, 512], F32, tag="ops")
            for w in range(N_WARM):
                nc.tensor.matmul(
                    wps[:, (w % 4) * P:(w % 4 + 1) * P], warm[:], warm[:],
                    start=(w % 4 == 0), stop=(w % 4 == 3))

            state = {}

            def emit_head(c):
                # transpose chunk c to feature-major + mm1 + softplus
                xin = xin_l[c]
                xT = xtpp.tile([P, KC, 512], F32R, tag="xT")
                for k in range(KC):
                    tps = tpsp.tile([P, 512], F32R, tag="tps")
                    for s in range(SC):
                        nc.tensor.matmul(
                            tps[:, s * P:(s + 1) * P],
                            xin[:, s, k * P:(k + 1) * P],
                            ident,
                            is_transpose=True,
                            start=(s == 0),
                            stop=(s == SC - 1),
                        )
                    nc.vector.tensor_copy(xT[:, k, :], tps[:].bitcast(F32))

                # mm1: D2 = [-Wd; +Wd] @ x^T [64, 512], rows 31/63 = 0 pad
                dps = dpsp.tile([64, 512], F32, tag="dps")
                for k in range(KC):
                    nc.tensor.matmul(
                        dps[:], wdT[k], xT[:, k, :],
                        start=(k == 0), stop=(k == KC - 1))

                # softplus blocks: t = exp(D2 + bias); sp = ln(t + 1)
                # (pad rows give ln2, nulled by the zero rows of A).
                t = actp.tile([64, 512], F32, tag="texp")
                nc.scalar.activation(t[:], dps[:], AF.Exp, bias=biasN)
                sp = spp.tile([64, 512], F32R, tag="sp")
                nc.scalar.activation(sp[:], t[:], AF.Ln, bias=1.0)
                state[c] = sp

            def emit_tail(c):
                # S = A @ sp (log path probs), C = exp(S)
                sp = state.pop(c)
                sps = spsp.tile([N_ALL, 512], F32, tag="sps")
                nc.tensor.matmul(sps[:], aT, sp[:], start=True, stop=True)
                cc = ccp.tile([N_ALL, 512], F32R, tag="cc")
                nc.scalar.activation(cc[:], sps[:], AF.Exp)

                # mm2: out^T chunk = leafG^T @ C  [128, 512]
                ops = opsp.tile([P, 512], F32, tag="ops")
                nc.tensor.matmul(ops[:], leafG, cc[:], start=True, stop=True)
                osb = osbp.tile([P, 512], F32, tag="osb")
                nc.scalar.copy(osb[:], ops[:])
                nc.sync.dma_start(y_d[:, c * 512:(c + 1) * 512], osb[:])

            # Software-pipelined emission: chunk c+1's transposes+mm1 go
            # to the PE queue before chunk c's mmA/mm2 so the PE never
            # stalls waiting on the ACT exp/ln chain.
            emit_head(0)
            for c in range(1, NCH):
                emit_head(c)
                emit_tail(c - 1)
            emit_tail(NCH - 1)
    return nc


_BUILD_CACHE = {}


def _pin_act_tables(nc):
    """Restrict Exp/Ln to the one table set that holds both, so the
    table-load placement pass emits a single ACT_TABLE_LOAD instead of
    thrashing between per-function sets (~2.7us per switch)."""
    from concourse import hw_specs
    tables = hw_specs.get_activation_tables(nc.m.arch)
    both = "natural_log_exp_and_others"
    if both in tables and AF.Exp in tables[both] and AF.Ln in tables[both]:
        for name, fns in tables.items():
            if name != both:
                fns.discard(AF.Exp)
                fns.discard(AF.Ln)


def build():
    if "nc" not in _BUILD_CACHE:
        nc = bacc.Bacc("TRN2", target_bir_lowering=False, debug=False,
                       num_devices=N_CORES)
        _pin_act_tables(nc)
        _emit(nc)
        nc.compile()
        _BUILD_CACHE["nc"] = nc
    return _BUILD_CACHE["nc"]


def host_prep(W, b, gamma, leaf):
    """Fold the tiny tree parameters into one packed constant block."""
    W = np.asarray(W, np.float32)
    b = np.asarray(b, np.float32)
    gamma = np.asarray(gamma, np.float32)
    leaf = np.asarray(leaf, np.float32)

    Wd = W[:, 0, :] - W[:, 1, :]                      # [31, 512]
    bd = b[:, 0] - b[:, 1]                            # [31]
    e = np.exp(gamma - gamma.max(-1, keepdims=True))
    g = e / e.sum(-1, keepdims=True)                  # [31, 2]

    path = np.zeros(N_ALL, np.float64)
    path[0] = 1.0
    for m in range(1, N_ALL):
        par = (m - 1) // 2
        path[m] = path[par] * g[par, 0]
    G = np.array([path[m] * (g[m, 1] if m < N_INT else 1.0)
                  for m in range(N_ALL)])
    leafG = (G[:, None] * leaf.astype(np.float64)).astype(np.float32)

    # A[row, m] = -1 if the edge lives on the root->m path.
    # Edge (node a, child j) -> row a (j=0) or row 32+a (j=1); rows 31/63 pad.
    A = np.zeros((64, N_ALL), np.float32)
    for m in range(N_ALL):
        node = m
        while node:
            par = (node - 1) // 2
            j = node - 2 * par - 1
            A[par if j == 0 else 32 + par, m] = -1.0
            node = par

    pk = np.zeros((P, PKW), np.float32)
    pk[:, 0:128] = np.eye(P, dtype=np.float32)
    # Doubled routing weights: cols 0..30 = -Wd^T, cols 32..62 = +Wd^T.
    wdTfull = np.ascontiguousarray(Wd.T)              # [512, 31]
    for k in range(KC):
        blk = wdTfull[k * P:(k + 1) * P]
        pk[:, 128 + 64 * k + 0:128 + 64 * k + N_INT] = -blk
        pk[:, 128 + 64 * k + 32:128 + 64 * k + 32 + N_INT] = blk
    pk[0:64, 384:447] = A
    pk[0:N_INT, 447] = -bd
    pk[32:32 + N_INT, 447] = bd
    pk[0:N_ALL, 448:576] = leafG
    return {"pk": pk}


def run(x, W, b, gamma, leaf, **spmd_kwargs):
    x = np.asarray(x, np.float32)
    consts = host_prep(W, b, gamma, leaf)
    shards = x.reshape(N_CORES, B_CORE, D_IN)
    in_maps = [dict(consts, x=np.ascontiguousarray(shards[i]))
               for i in range(N_CORES)]
    nc = build()
    res = run_bass_kernel_spmd(nc, in_maps, list(range(N_CORES)), **spmd_kwargs)
    y = np.concatenate(
        [np.ascontiguousarray(res.results[i]["y"].T) for i in range(N_CORES)],
        axis=0)
    return y, res


def kernel(x, W, b, gamma, leaf):
    y, _ = run(x, W, b, gamma, leaf)
    return y


# revision 9
# speedup vs baseline: 1.4671x; 1.0284x over previous
"""Trainium2 Bass kernel for nn_MixNode (soft decision tree / MoE routing).

The recursive MixNode tree collapses algebraically:
    out[b] = sum_m C_m(x_b) * leafG[m]
where leafG folds the (input-independent) gamma-softmax products into the
leaf vectors, and C_m = prod of routing probabilities along the root->m
path. With delta = logit0 - logit1 per internal node, the two routing
probs are sigmoid(+-delta), so log C = A @ (-softplus(-+delta)) for a
constant 0/-1 path matrix A. softplus(z) = ln(exp(z) + 1) is computed as
Exp then Ln(x + 1); both +-delta blocks are produced by one doubled
matmul (weights [-Wd; +Wd]) so each chunk needs only one Exp + one Ln.

Per core (batch shard 2048 rows = 4 chunks x 512, loaded as 8 x 256-row
half-chunks so the PE starts ~1.5us earlier and the last-chunk tail is
short):
    DMA x -> PE transpose (fp32r, accumulated in k-pair PSUM banks) ->
    DVE cast to SBUF -> mm1 D2 = [-Wd;+Wd] @ x^T (per half) ->
    ACT exp -> ACT ln1p -> mmA S = A @ sp -> ACT exp -> C ->
    mm2 out^T = leafG^T @ C -> ACT copy -> DMA out.
All matmuls run in float32r (full-rate PE; ~1e-4 rel err, far under the
2e-2 gate). The output is produced output-major [128, 2048] per core so
mm2 streams 512 columns per chunk and the store DMA uses 2 KiB
descriptors; the host transposes when gathering. Exp/Ln are pinned to
the one ACT table set holding both (single table load).

Engine budget per 512-chunk (~2.9us DMA cadence): PE ~2.5us
(16 transposes + 8 mm1 + mmA + mm2), DVE ~2.7us (4 x 512-free casts),
ACT ~2.75us (exp+ln+exp + PSUM->SBUF out copy). The last chunk's tail
(mmA/exp/mm2/copy/store) runs at half width to shorten the critical
path after the final DMA arrival.

Sharding: pure data parallelism over the batch dim across 8 cores;
the small tree parameters are folded host-side and replicated.
"""

import os
import sys

import numpy as np

for _p in ("/opt/trn_rl_repo", "/root/.axon_site/_ro/trn_rl_repo"):
    if os.path.isdir(_p) and _p not in sys.path:
        sys.path.append(_p)

import concourse.tile as tile
from concourse import bacc, mybir
from concourse.bass_utils import run_bass_kernel_spmd

N_CORES = 8
BATCH, D_IN, D_OUT = 16384, 512, 128
B_CORE = BATCH // N_CORES  # 2048
N_INT, N_ALL = 31, 63
P = 128
NCH = 4  # batch chunks per core, 512 rows each
SC = 4   # 128-row subtiles per chunk
KC = 4   # 128-feature chunks
PKW = 128 + 256 + 63 + 1 + 128  # packed const width = 576

F32 = mybir.dt.float32
F32R = mybir.dt.float32r
AF = mybir.ActivationFunctionType

N_WARM = 7  # PE clock-ramp matmuls covering the first x DMA latency


def _emit(nc):
    x_d = nc.dram_tensor("x", [B_CORE, D_IN], F32R, kind="ExternalInput")
    pk_d = nc.dram_tensor("pk", [P, PKW], F32R, kind="ExternalInput")
    y_d = nc.dram_tensor("y", [D_OUT, B_CORE], F32, kind="ExternalOutput")

    with tile.TileContext(nc) as tc:
        with (
            tc.tile_pool(name="const", bufs=1) as constp,
            tc.tile_pool(name="xin", bufs=4) as xinp,
            tc.tile_pool(name="xtp", bufs=2) as xtpp,
            tc.tile_pool(name="act", bufs=2) as actp,
            tc.tile_pool(name="spc", bufs=2) as spp,
            tc.tile_pool(name="ccp", bufs=2) as ccp,
            tc.tile_pool(name="osbp", bufs=2) as osbp,
            tc.tile_pool(name="tps", bufs=3, space="PSUM") as tpsp,
            tc.tile_pool(name="dps", bufs=2, space="PSUM") as dpsp,
            tc.tile_pool(name="sps", bufs=2, space="PSUM") as spsp,
            tc.tile_pool(name="ops", bufs=1, space="PSUM") as opsp,
        ):
            # Prefetch the full x shard: one DMA per 256-row half-chunk on
            # the sync queue (256 descriptors of 2 KiB), issued up front.
            xin_l = []
            for c in range(NCH):
                xin = xinp.tile([P, SC, D_IN], F32R, tag="xin")
                for h in range(2):
                    src = x_d[c * 512 + h * 256:c * 512 + (h + 1) * 256, :]
                    nc.sync.dma_start(
                        xin[:, 2 * h:2 * h + 2, :],
                        src.rearrange("(s p) f -> p s f", p=P))
                xin_l.append(xin)

            # Packed constants in a single DMA on the scalar queue. The
            # tile is float32r because the BIR verifier requires fp32r
            # matmul inputs to be *produced* as fp32r; the ACT bias slice
            # is bitcast back (same bits either way).
            pk = constp.tile([P, PKW], F32R)
            nc.scalar.dma_start(pk[:], pk_d[:])
            ident = pk[:, 0:128]
            wdT = [pk[:, 128 + 64 * k:128 + 64 * (k + 1)]
                   for k in range(KC)]
            aT = pk[0:64, 384:447]
            biasN = pk[0:64, 447:448].bitcast(F32)
            leafG = pk[0:63, 448:576]

            # PE warm-up: dummy fp32 matmuls on a zeroed scratch tile flip
            # the HAM clock gate toward 2.4GHz and keep the PE busy until
            # the first x half-chunk lands.
            warm = constp.tile([P, P], F32)
            nc.gpsimd.memset(warm[:], 0.0)
            wps = tpsp.tile([P, 2, 256], F32, tag="tps")
            for w in range(N_WARM):
                nc.tensor.matmul(
                    wps[:, w % 2, (w % 2) * P:(w % 2 + 1) * P],
                    warm[:], warm[:],
                    start=True, stop=True)

            state = {}

            def emit_head(c):
                # One 256-row half at a time: transpose to feature-major
                # (k-pairs share a PSUM bank -> one 512-free DVE cast),
                # then mm1 into this half's dps columns.
                xin = xin_l[c]
                xT = xtpp.tile([P, KC, 512], F32R, tag="xT")
                dps = dpsp.tile([64, 512], F32, tag="dps")
                for h in range(2):
                    cols = slice(h * 256, (h + 1) * 256)
                    for kp in range(2):
                        tps = tpsp.tile([P, 2, 256], F32R, tag="tps")
                        for k2 in range(2):
                            k = 2 * kp + k2
                            for s2 in range(2):
                                s = 2 * h + s2
                                nc.tensor.matmul(
                                    tps[:, k2, s2 * P:(s2 + 1) * P],
                                    xin[:, s, k * P:(k + 1) * P],
                                    ident,
                                    is_transpose=True,
                                    start=(s2 == 0),
                                    stop=(s2 == 1),
                                )
                        nc.vector.tensor_copy(
                            xT[:, 2 * kp:2 * kp + 2, cols],
                            tps[:].bitcast(F32))
                    for k in range(KC):
                        nc.tensor.matmul(
                            dps[:, cols], wdT[k], xT[:, k, cols],
                            start=(k == 0), stop=(k == KC - 1))

                # softplus blocks: t = exp(D2 + bias); sp = ln(t + 1)
                # (pad rows give ln2, nulled by the zero rows of A).
                t = actp.tile([64, 512], F32, tag="texp")
                nc.scalar.activation(t[:], dps[:], AF.Exp, bias=biasN)
                sp = spp.tile([64, 512], F32R, tag="sp")
                nc.scalar.activation(sp[:], t[:], AF.Ln, bias=1.0)
                state[c] = sp

            def emit_tail(c, split):
                # S = A @ sp, C = exp(S), out^T = leafG^T @ C, store.
                # The last chunk runs this at half width to shorten the
                # serial tail after the final DMA arrival.
                sp = state.pop(c)
                for h in range(2 if split else 1):
                    w = 256 if split else 512
                    cols = slice(h * w, (h + 1) * w)
                    sps = spsp.tile([N_ALL, w], F32, tag="sps")
                    nc.tensor.matmul(sps[:], aT, sp[:, cols],
                                     start=True, stop=True)
                    cc = ccp.tile([N_ALL, w], F32R, tag="cc")
                    nc.scalar.activation(cc[:], sps[:], AF.Exp)
                    ops = opsp.tile([P, w], F32, tag="ops")
                    nc.tensor.matmul(ops[:], leafG, cc[:],
                                     start=True, stop=True)
                    osb = osbp.tile([P, w], F32, tag="osb")
                    nc.scalar.copy(osb[:], ops[:])
                    nc.sync.dma_start(
                        y_d[:, c * 512 + h * w:c * 512 + (h + 1) * w],
                        osb[:])

            # Software-pipelined emission: chunk c+1's transposes+mm1 go
            # to the PE queue before chunk c's mmA/mm2 so the PE never
            # stalls waiting on the ACT exp/ln chain.
            emit_head(0)
            for c in range(1, NCH):
                emit_head(c)
                emit_tail(c - 1, split=False)
            emit_tail(NCH - 1, split=True)
    return nc


_BUILD_CACHE = {}


def _pin_act_tables(nc):
    """Restrict Exp/Ln to the one table set that holds both, so the
    table-load placement pass emits a single ACT_TABLE_LOAD instead of
    thrashing between per-function sets (~2.7us per switch)."""
    from concourse import hw_specs
    tables = hw_specs.get_activation_tables(nc.m.arch)
    both = "natural_log_exp_and_others"
    if both in tables and AF.Exp in tables[both] and AF.Ln in tables[both]:
        for name, fns in tables.items():
            if name != both:
                fns.discard(AF.Exp)
                fns.discard(AF.Ln)


def build():
    if "nc" not in _BUILD_CACHE:
        nc = bacc.Bacc("TRN2", target_bir_lowering=False, debug=False,
                       num_devices=N_CORES)
        _pin_act_tables(nc)
        _emit(nc)
        nc.compile()
        _BUILD_CACHE["nc"] = nc
    return _BUILD_CACHE["nc"]


def host_prep(W, b, gamma, leaf):
    """Fold the tiny tree parameters into one packed constant block."""
    W = np.asarray(W, np.float32)
    b = np.asarray(b, np.float32)
    gamma = np.asarray(gamma, np.float32)
    leaf = np.asarray(leaf, np.float32)

    Wd = W[:, 0, :] - W[:, 1, :]                      # [31, 512]
    bd = b[:, 0] - b[:, 1]                            # [31]
    e = np.exp(gamma - gamma.max(-1, keepdims=True))
    g = e / e.sum(-1, keepdims=True)                  # [31, 2]

    path = np.zeros(N_ALL, np.float64)
    path[0] = 1.0
    for m in range(1, N_ALL):
        par = (m - 1) // 2
        path[m] = path[par] * g[par, 0]
    G = np.array([path[m] * (g[m, 1] if m < N_INT else 1.0)
                  for m in range(N_ALL)])
    leafG = (G[:, None] * leaf.astype(np.float64)).astype(np.float32)

    # A[row, m] = -1 if the edge lives on the root->m path.
    # Edge (node a, child j) -> row a (j=0) or row 32+a (j=1); rows 31/63 pad.
    A = np.zeros((64, N_ALL), np.float32)
    for m in range(N_ALL):
        node = m
        while node:
            par = (node - 1) // 2
            j = node - 2 * par - 1
            A[par if j == 0 else 32 + par, m] = -1.0
            node = par

    pk = np.zeros((P, PKW), np.float32)
    pk[:, 0:128] = np.eye(P, dtype=np.float32)
    # Doubled routing weights: cols 0..30 = -Wd^T, cols 32..62 = +Wd^T.
    wdTfull = np.ascontiguousarray(Wd.T)              # [512, 31]
    for k in range(KC):
        blk = wdTfull[k * P:(k + 1) * P]
        pk[:, 128 + 64 * k + 0:128 + 64 * k + N_INT] = -blk
        pk[:, 128 + 64 * k + 32:128 + 64 * k + 32 + N_INT] = blk
    pk[0:64, 384:447] = A
    pk[0:N_INT, 447] = -bd
    pk[32:32 + N_INT, 447] = bd
    pk[0:N_ALL, 448:576] = leafG
    return {"pk": pk}


def run(x, W, b, gamma, leaf, **spmd_kwargs):
    x = np.asarray(x, np.float32)
    consts = host_prep(W, b, gamma, leaf)
    shards = x.reshape(N_CORES, B_CORE, D_IN)
    in_maps = [dict(consts, x=np.ascontiguousarray(shards[i]))
               for i in range(N_CORES)]
    nc = build()
    res = run_bass_kernel_spmd(nc, in_maps, list(range(N_CORES)), **spmd_kwargs)
    y = np.concatenate(
        [np.ascontiguousarray(res.results[i]["y"].T) for i in range(N_CORES)],
        axis=0)
    return y, res


def kernel(x, W, b, gamma, leaf):
    y, _ = run(x, W, b, gamma, leaf)
    return y


# revision 13
# speedup vs baseline: 2.0880x; 1.4232x over previous
"""Trainium2 Bass kernel for nn_MixNode (soft decision tree / MoE routing).

The recursive MixNode tree collapses algebraically:
    out[b] = sum_m C_m(x_b) * leafG[m]
where leafG folds the (input-independent) gamma-softmax products into the
leaf vectors, and C_m = prod of routing probabilities along the root->m
path. With delta = logit0 - logit1 per internal node, the two routing
probs are sigmoid(+-delta), so log C = A @ (-softplus(-+delta)) for a
constant 0/-1 path matrix A. softplus(z) = ln(exp(z) + 1) is computed as
Exp then Ln(x + 1); both +-delta blocks are produced by one doubled
matmul (weights [-Wd; +Wd]) so each chunk needs only one Exp + one Ln.

Per core (batch shard 2048 rows = 4 chunks x 512, loaded as 8 x 256-row
half-chunks so the PE starts ~1.5us earlier and the last-chunk tail is
short):
    DMA x -> PE transpose (fp32r, accumulated in k-pair PSUM banks) ->
    DVE cast to SBUF -> mm1 D2 = [-Wd;+Wd] @ x^T (per half) ->
    ACT exp -> ACT ln1p -> mmA S = A @ sp -> ACT exp -> C ->
    mm2 out^T = leafG^T @ C -> ACT copy -> DMA out.
All matmuls run in float32r (full-rate PE; ~1e-4 rel err, far under the
2e-2 gate). The output is produced output-major [128, 2048] per core so
mm2 streams 512 columns per chunk and the store DMA uses 2 KiB
descriptors; the host transposes when gathering. Exp/Ln are pinned to
the one ACT table set holding both (single table load).

Engine budget per 512-chunk (~2.9us DMA cadence): PE ~2.5us
(16 transposes + 8 mm1 + mmA + mm2), DVE ~2.7us (4 x 512-free casts),
ACT ~2.75us (exp+ln+exp + PSUM->SBUF out copy). The last chunk's tail
(mmA/exp/mm2/copy/store) runs at half width to shorten the critical
path after the final DMA arrival.

Sharding: pure data parallelism over the batch dim across 8 cores;
the small tree parameters are folded host-side and replicated.
"""

import os
import sys

import numpy as np

for _p in ("/opt/trn_rl_repo", "/root/.axon_site/_ro/trn_rl_repo"):
    if os.path.isdir(_p) and _p not in sys.path:
        sys.path.append(_p)

import concourse.tile as tile
from concourse import bacc, mybir
from concourse.bass_utils import run_bass_kernel_spmd

N_CORES = 8
BATCH, D_IN, D_OUT = 16384, 512, 128
B_CORE = BATCH // N_CORES  # 2048
N_INT, N_ALL = 31, 63
P = 128
NCH = 4  # batch chunks per core, 512 rows each
SC = 4   # 128-row subtiles per chunk
KC = 4   # 128-feature chunks
PKW = 128 + 256 + 63 + 1 + 128  # packed const width = 576

F32 = mybir.dt.float32
F32R = mybir.dt.float32r
AF = mybir.ActivationFunctionType

N_WARM = 6  # PE clock-ramp matmuls covering the first x DMA latency


def _emit(nc):
    x_d = nc.dram_tensor("x", [B_CORE, D_IN], F32R, kind="ExternalInput")
    pk_d = nc.dram_tensor("pk", [P, PKW], F32R, kind="ExternalInput")
    y_d = nc.dram_tensor("y", [D_OUT, B_CORE], F32, kind="ExternalOutput")

    with tile.TileContext(nc) as tc:
        with (
            tc.tile_pool(name="const", bufs=1) as constp,
            tc.tile_pool(name="xin", bufs=4) as xinp,
            tc.tile_pool(name="xtp", bufs=2) as xtpp,
            tc.tile_pool(name="act", bufs=2) as actp,
            tc.tile_pool(name="spc", bufs=2) as spp,
            tc.tile_pool(name="ccp", bufs=2) as ccp,
            tc.tile_pool(name="osbp", bufs=2) as osbp,
            tc.tile_pool(name="tps", bufs=2, space="PSUM") as tpsp,
            tc.tile_pool(name="dps", bufs=2, space="PSUM") as dpsp,
            tc.tile_pool(name="sps", bufs=1, space="PSUM") as spsp,
            tc.tile_pool(name="ops", bufs=1, space="PSUM") as opsp,
        ):
            # Prefetch the full x shard: one DMA per 256-row half-chunk on
            # the sync queue (256 descriptors of 2 KiB), issued up front.
            xin_l = []
            for c in range(NCH):
                xin = xinp.tile([P, SC, D_IN], F32R, tag="xin")
                for h in range(2):
                    src = x_d[c * 512 + h * 256:c * 512 + (h + 1) * 256, :]
                    nc.sync.dma_start(
                        xin[:, 2 * h:2 * h + 2, :],
                        src.rearrange("(s p) f -> p s f", p=P))
                xin_l.append(xin)

            # Packed constants in a single DMA on the scalar queue. The
            # tile is float32r because the BIR verifier requires fp32r
            # matmul inputs to be *produced* as fp32r; the ACT bias slice
            # is bitcast back (same bits either way).
            pk = constp.tile([P, PKW], F32R)
            nc.scalar.dma_start(pk[:], pk_d[:])
            ident = pk[:, 0:128]
            wdT = [pk[:, 128 + 64 * k:128 + 64 * (k + 1)]
                   for k in range(KC)]
            aT = pk[0:64, 384:447]
            biasN = pk[0:64, 447:448].bitcast(F32)
            leafG = pk[0:63, 448:576]

            # PE warm-up: dummy fp32 matmuls on a zeroed scratch tile flip
            # the HAM clock gate toward 2.4GHz and keep the PE busy until
            # the first x half-chunk lands.
            warm = constp.tile([P, P], F32)
            nc.gpsimd.memset(warm[:], 0.0)
            wps = tpsp.tile([P, KC, 256], F32, tag="tps")
            for w in range(N_WARM):
                nc.tensor.matmul(
                    wps[:, w % 4, (w % 2) * P:(w % 2 + 1) * P],
                    warm[:], warm[:],
                    start=True, stop=True)

            state = {}

            def emit_head(c):
                # One 256-row half at a time: transpose to feature-major
                # (k-pairs share a PSUM bank -> one 512-free DVE cast),
                # then mm1 into this half's dps columns.
                xin = xin_l[c]
                xT = xtpp.tile([P, KC, 512], F32R, tag="xT")
                dps = dpsp.tile([64, 512], F32, tag="dps")
                for h in range(2):
                    cols = slice(h * 256, (h + 1) * 256)
                    tps = tpsp.tile([P, KC, 256], F32R, tag="tps")
                    for k in range(KC):
                        for s2 in range(2):
                            s = 2 * h + s2
                            nc.tensor.matmul(
                                tps[:, k, s2 * P:(s2 + 1) * P],
                                xin[:, s, k * P:(k + 1) * P],
                                ident,
                                is_transpose=True,
                                start=(s2 == 0),
                                stop=(s2 == 1),
                            )
                    nc.vector.tensor_copy(
                        xT[:, :, cols], tps[:].bitcast(F32))
                for k in range(KC):
                    nc.tensor.matmul(
                        dps[:], wdT[k], xT[:, k, :],
                        start=(k == 0), stop=(k == KC - 1))

                # softplus blocks: t = exp(D2 + bias); sp = ln(t + 1)
                # (pad rows give ln2, nulled by the zero rows of A).
                t = actp.tile([64, 512], F32, tag="texp")
                nc.scalar.activation(t[:], dps[:], AF.Exp, bias=biasN)
                sp = spp.tile([64, 512], F32R, tag="sp")
                nc.scalar.activation(sp[:], t[:], AF.Ln, bias=1.0)
                state[c] = sp

            def emit_tail(c, split):
                # S = A @ sp, C = exp(S), out^T = leafG^T @ C, store.
                # The last chunk runs this at half width to shorten the
                # serial tail after the final DMA arrival.
                sp = state.pop(c)
                for h in range(2 if split else 1):
                    w = 256 if split else 512
                    cols = slice(h * w, (h + 1) * w)
                    sps = spsp.tile([N_ALL, w], F32, tag="sps")
                    nc.tensor.matmul(sps[:], aT, sp[:, cols],
                                     start=True, stop=True)
                    cc = ccp.tile([N_ALL, w], F32R, tag="cc")
                    nc.scalar.activation(cc[:], sps[:], AF.Exp)
                    ops = opsp.tile([P, w], F32, tag="ops")
                    nc.tensor.matmul(ops[:], leafG, cc[:],
                                     start=True, stop=True)
                    osb = osbp.tile([P, w], F32, tag="osb")
                    nc.scalar.copy(osb[:], ops[:])
                    nc.sync.dma_start(
                        y_d[:, c * 512 + h * w:c * 512 + (h + 1) * w],
                        osb[:])

            # Software-pipelined emission: chunk c+1's transposes+mm1 go
            # to the PE queue before chunk c's mmA/mm2 so the PE never
            # stalls waiting on the ACT exp/ln chain.
            emit_head(0)
            for c in range(1, NCH):
                emit_head(c)
                emit_tail(c - 1, split=False)
            emit_tail(NCH - 1, split=True)
    return nc


_BUILD_CACHE = {}


def _pin_act_tables(nc):
    """Restrict Exp/Ln to the one table set that holds both, so the
    table-load placement pass emits a single ACT_TABLE_LOAD instead of
    thrashing between per-function sets (~2.7us per switch)."""
    from concourse import hw_specs
    tables = hw_specs.get_activation_tables(nc.m.arch)
    both = "natural_log_exp_and_others"
    if both in tables and AF.Exp in tables[both] and AF.Ln in tables[both]:
        for name, fns in tables.items():
            if name != both:
                fns.discard(AF.Exp)
                fns.discard(AF.Ln)


def build():
    if "nc" not in _BUILD_CACHE:
        nc = bacc.Bacc("TRN2", target_bir_lowering=False, debug=False,
                       num_devices=N_CORES)
        _pin_act_tables(nc)
        _emit(nc)
        nc.compile()
        _BUILD_CACHE["nc"] = nc
    return _BUILD_CACHE["nc"]


def host_prep(W, b, gamma, leaf):
    """Fold the tiny tree parameters into one packed constant block."""
    W = np.asarray(W, np.float32)
    b = np.asarray(b, np.float32)
    gamma = np.asarray(gamma, np.float32)
    leaf = np.asarray(leaf, np.float32)

    Wd = W[:, 0, :] - W[:, 1, :]                      # [31, 512]
    bd = b[:, 0] - b[:, 1]                            # [31]
    e = np.exp(gamma - gamma.max(-1, keepdims=True))
    g = e / e.sum(-1, keepdims=True)                  # [31, 2]

    path = np.zeros(N_ALL, np.float64)
    path[0] = 1.0
    for m in range(1, N_ALL):
        par = (m - 1) // 2
        path[m] = path[par] * g[par, 0]
    G = np.array([path[m] * (g[m, 1] if m < N_INT else 1.0)
                  for m in range(N_ALL)])
    leafG = (G[:, None] * leaf.astype(np.float64)).astype(np.float32)

    # A[row, m] = -1 if the edge lives on the root->m path.
    # Edge (node a, child j) -> row a (j=0) or row 32+a (j=1); rows 31/63 pad.
    A = np.zeros((64, N_ALL), np.float32)
    for m in range(N_ALL):
        node = m
        while node:
            par = (node - 1) // 2
            j = node - 2 * par - 1
            A[par if j == 0 else 32 + par, m] = -1.0
            node = par

    pk = np.zeros((P, PKW), np.float32)
    pk[:, 0:128] = np.eye(P, dtype=np.float32)
    # Doubled routing weights: cols 0..30 = -Wd^T, cols 32..62 = +Wd^T.
    wdTfull = np.ascontiguousarray(Wd.T)              # [512, 31]
    for k in range(KC):
        blk = wdTfull[k * P:(k + 1) * P]
        pk[:, 128 + 64 * k + 0:128 + 64 * k + N_INT] = -blk
        pk[:, 128 + 64 * k + 32:128 + 64 * k + 32 + N_INT] = blk
    pk[0:64, 384:447] = A
    pk[0:N_INT, 447] = -bd
    pk[32:32 + N_INT, 447] = bd
    pk[0:N_ALL, 448:576] = leafG
    return {"pk": pk}


def run(x, W, b, gamma, leaf, **spmd_kwargs):
    x = np.asarray(x, np.float32)
    consts = host_prep(W, b, gamma, leaf)
    shards = x.reshape(N_CORES, B_CORE, D_IN)
    in_maps = [dict(consts, x=np.ascontiguousarray(shards[i]))
               for i in range(N_CORES)]
    nc = build()
    res = run_bass_kernel_spmd(nc, in_maps, list(range(N_CORES)), **spmd_kwargs)
    y = np.concatenate(
        [np.ascontiguousarray(res.results[i]["y"].T) for i in range(N_CORES)],
        axis=0)
    return y, res


def kernel(x, W, b, gamma, leaf):
    y, _ = run(x, W, b, gamma, leaf)
    return y
